# revision 6
# baseline (speedup 1.0000x reference)
"""DescriptorMatcher (SNN ratio-test) Trainium2 kernel.

Problem: desc1, desc2 (8192, 128) f32. For each row i of desc1: find the two
nearest neighbours in desc2 (L2), ratio = d1/d2, mask = ratio <= 0.8, emit
(ratio-or-0, [i, argmin]-or--1, mask).

Sharding: desc1 rows split across 8 cores (1024 rows each), desc2 replicated.

Per-core algorithm (all on device):
  - Build bf16 hi/lo splits of d1-slice^T and desc2^T.
  - s = 2*d1@d2^T - ||d2||^2  (negated squared distance + per-row const),
    computed per (128 rows x 512 cols) chunk as 3 bf16 matmul passes
    (hi*hi + hi*lo + lo*hi, scaled by 2 via pre-scaled d1 tiles) plus a K=2
    "c-fold" matmul adding -||d2||^2 (split hi/lo in bf16).
  - ScalarE copies PSUM chunks into an SBUF row-tile s (128 x 8192).
  - VectorE max8 -> top-8 values; find_index8 -> their column indices.
  - Exact refinement: indirect-DMA gather desc2[j1], desc2[j2]; recompute
    both distances exactly in fp32; ratio = sqrt(d1sq/d2sq) (ACT sqrt +
    one Newton step); mask; masked outputs.
Outputs per core: ratio (128,8) f32, idx (128,8) i32, mask (128,8) u8,
column rt = row-tile rt (rows rt*128..rt*128+127 of the core's slice).
"""

import numpy as np

import concourse.bass as bass
import concourse.mybir as mybir
from concourse.bacc import Bacc
from concourse.tile import TileContext
from concourse.bass_utils import run_bass_kernel_spmd

P = 128          # partitions / rows per row-tile
B1 = 8192        # rows of desc1
B2 = 8192        # rows of desc2
D = 128          # descriptor dim
NCORES = 8
ROWS_PER_CORE = B1 // NCORES       # 1024
RT = ROWS_PER_CORE // P            # 8 row-tiles per core
N = 512                            # matmul free-dim chunk
NCH = B2 // N                      # 16 chunks
TH = 0.8

F32 = mybir.dt.float32
BF16 = mybir.dt.bfloat16
I32 = mybir.dt.int32
U8 = mybir.dt.uint8
U32 = mybir.dt.uint32
AF = mybir.ActivationFunctionType
OP = mybir.AluOpType

_CACHE = {}


def build_nc(stage=99, nrt=RT):
    nc = Bacc()
    d1n = nc.dram_tensor("d1n", [ROWS_PER_CORE, D], F32, kind="ExternalInput")
    d1t = nc.dram_tensor("d1t", [D, ROWS_PER_CORE], F32, kind="ExternalInput")
    d2t = nc.dram_tensor("d2t", [D, B2], F32, kind="ExternalInput")
    d2n = nc.dram_tensor("d2n", [B2, D], F32, kind="ExternalInput")
    o_ratio = nc.dram_tensor("o_ratio", [P, RT], F32, kind="ExternalOutput")
    o_idx = nc.dram_tensor("o_idx", [P, RT], I32, kind="ExternalOutput")
    o_mask = nc.dram_tensor("o_mask", [P, RT], U8, kind="ExternalOutput")

    with TileContext(nc) as tc:
        with tc.tile_pool(name="const", bufs=1) as cpool, \
             tc.tile_pool(name="load", bufs=3) as ldpool, \
             tc.tile_pool(name="setup", bufs=3) as supool, \
             tc.tile_pool(name="scan", bufs=2) as spool, \
             tc.tile_pool(name="small", bufs=3) as smpool, \
             tc.tile_pool(name="ps_set", bufs=2, space="PSUM") as psB, \
             tc.tile_pool(name="ps_mm", bufs=5, space="PSUM") as psA:

            # ---------------- constants / persistent tiles ----------------
            d2hi = cpool.tile([P, B2], BF16, tag="d2hi")
            d2lo = cpool.tile([P, B2], BF16, tag="d2lo")
            cf = cpool.tile([2, B2], BF16, tag="cf")       # rows: -d2sq hi, lo
            d1hi = cpool.tile([P, ROWS_PER_CORE], BF16, tag="d1hi")
            d1lo = cpool.tile([P, ROWS_PER_CORE], BF16, tag="d1lo")
            mones = cpool.tile([P, P], F32, tag="mones")   # -1.0 weights
            ones2 = cpool.tile([2, P], BF16, tag="ones2")
            outr = cpool.tile([P, RT], F32, tag="outr")
            outi = cpool.tile([P, RT], F32, tag="outi")    # staged as f32
            outm = cpool.tile([P, RT], F32, tag="outm")

            nc.gpsimd.memset(mones, -1.0)
            nc.gpsimd.memset(ones2, 1.0)
            nc.gpsimd.memset(outr, 0.0)
            nc.gpsimd.memset(outi, -1.0)
            nc.gpsimd.memset(outm, 0.0)

            # ---------------- setup: d1 splits (x2 for the "2*" in 2ab) ----
            d1f = ldpool.tile([P, ROWS_PER_CORE], F32, tag="d1f")
            nc.sync.dma_start(out=d1f, in_=d1t[:, :])
            d1x2 = supool.tile([P, ROWS_PER_CORE], F32, tag="d1x2")
            nc.vector.tensor_scalar(d1x2, d1f, 2.0, scalar2=None, op0=OP.mult)
            nc.scalar.copy(d1hi, d1x2)
            d1lo32 = supool.tile([P, ROWS_PER_CORE], F32, tag="d1lo32")
            nc.gpsimd.tensor_sub(d1lo32, d1x2, d1hi)
            nc.gpsimd.tensor_copy(d1lo, d1lo32)

            # ---------------- setup: d2 splits + c rows, per chunk ---------
            for n in range(NCH):
                sl = bass.ts(n, N)
                d2c = ldpool.tile([P, N], F32, tag="d2c")
                nc.sync.dma_start(out=d2c, in_=d2t[:, sl])
                nc.scalar.copy(d2hi[:, sl], d2c)
                d2lo32 = supool.tile([P, N], F32, tag="d2lo32")
                nc.gpsimd.tensor_sub(d2lo32, d2c, d2hi[:, sl])
                nc.gpsimd.tensor_copy(d2lo[:, sl], d2lo32)
                # c row: -sum(d2^2) replicated over partitions via matmul
                sq = supool.tile([P, N], F32, tag="sq")
                nc.scalar.activation(sq, d2c, AF.Square)
                cps = psB.tile([P, N], F32, tag="csq")
                nc.tensor.matmul(cps, mones, sq, start=True, stop=True)
                nc.scalar.copy(cf[0:1, sl], cps[0:1, :])
                crep = supool.tile([1, N], F32, tag="crep")
                nc.scalar.copy(crep, cps[0:1, :])
                crem = supool.tile([1, N], F32, tag="crem")
                nc.gpsimd.tensor_sub(crem, crep, cf[0:1, sl])
                clo = supool.tile([1, N], BF16, tag="clo")
                nc.gpsimd.tensor_copy(clo, crem)
                # compute engines cannot write at partition base 1 -> DMA it
                nc.sync.dma_start(out=cf[1:2, sl], in_=clo)

            # ---------------- steady: per row-tile --------------------------
            for rt in range(nrt if stage >= 2 else 0):
                rsl = bass.ts(rt, P)
                l_hi = d1hi[:, rsl]
                l_lo = d1lo[:, rsl]
                d1nc = ldpool.tile([P, D], F32, tag="d1nc")
                nc.sync.dma_start(out=d1nc, in_=d1n[rsl, :])

                s_tile = spool.tile([P, B2], F32, tag="s")
                for n in range(NCH):
                    sl = bass.ts(n, N)
                    ps = psA.tile([P, N], F32, tag="mm")
                    nc.tensor.matmul(ps, l_hi, d2hi[:, sl], start=True, stop=False)
                    nc.tensor.matmul(ps, l_hi, d2lo[:, sl], start=False, stop=False)
                    nc.tensor.matmul(ps, l_lo, d2hi[:, sl], start=False, stop=False)
                    nc.tensor.matmul(ps, ones2, cf[:, sl], start=False, stop=True)
                    nc.scalar.copy(s_tile[:, sl], ps)

                if stage < 3:
                    continue
                m8 = smpool.tile([P, 8], F32, tag="m8")
                nc.vector.max(out=m8, in_=s_tile)
                i8 = smpool.tile([P, 8], U32, tag="i8")
                nc.vector.max_index(out=i8, in_max=m8, in_values=s_tile)

                if stage < 4:
                    continue
                # exact refinement: gather desc2 rows for top-2 indices
                g1 = smpool.tile([P, D], F32, tag="g1")
                nc.gpsimd.indirect_dma_start(
                    out=g1, out_offset=None, in_=d2n[:, :],
                    in_offset=bass.IndirectOffsetOnAxis(ap=i8[:, 0:1], axis=0))
                g2 = smpool.tile([P, D], F32, tag="g2")
                nc.gpsimd.indirect_dma_start(
                    out=g2, out_offset=None, in_=d2n[:, :],
                    in_offset=bass.IndirectOffsetOnAxis(ap=i8[:, 1:2], axis=0))

                if stage < 5:
                    continue
                t1 = smpool.tile([P, D], F32, tag="t1")
                nc.gpsimd.tensor_sub(t1, g1, d1nc)
                t2 = smpool.tile([P, D], F32, tag="t2")
                nc.gpsimd.tensor_sub(t2, g2, d1nc)
                sc1 = smpool.tile([P, D], F32, tag="sc1")
                nc.gpsimd.tensor_mul(sc1, t1, t1)
                dist1 = smpool.tile([P, 1], F32, tag="dist1")
                nc.vector.reduce_sum(dist1, sc1, axis=mybir.AxisListType.X)
                sc2 = smpool.tile([P, D], F32, tag="sc2")
                nc.gpsimd.tensor_mul(sc2, t2, t2)
                dist2 = smpool.tile([P, 1], F32, tag="dist2")
                nc.vector.reduce_sum(dist2, sc2, axis=mybir.AxisListType.X)

                # ratio = sqrt(dist1/dist2), one Newton step for the sqrt
                if stage < 6:
                    continue
                rec = smpool.tile([P, 1], F32, tag="rec")
                nc.vector.reciprocal(rec, dist2)
                q = smpool.tile([P, 1], F32, tag="q")
                nc.vector.tensor_mul(q, dist1, rec)
                if stage < 7:
                    continue
                r0 = smpool.tile([P, 1], F32, tag="r0")
                nc.scalar.activation(r0, q, AF.Sqrt)
                rr = smpool.tile([P, 1], F32, tag="rr")
                nc.vector.reciprocal(rr, r0)
                t = smpool.tile([P, 1], F32, tag="t")
                nc.vector.tensor_mul(t, q, rr)
                r1 = smpool.tile([P, 1], F32, tag="r1")
                nc.vector.tensor_add(r1, r0, t)
                nc.vector.tensor_scalar(r1, r1, 0.5, scalar2=None, op0=OP.mult)

                if stage < 8:
                    continue
                maskf = smpool.tile([P, 1], F32, tag="maskf")
                nc.vector.tensor_scalar(maskf, r1, TH, scalar2=None, op0=OP.is_le)
                nc.vector.tensor_mul(outr[:, rt:rt + 1], r1, maskf)
                nc.vector.tensor_copy(outm[:, rt:rt + 1], maskf)

                if stage < 9:
                    continue
                jf = smpool.tile([P, 1], F32, tag="jf")
                nc.vector.tensor_copy(jf, i8[:, 0:1])
                nc.vector.tensor_scalar(jf, jf, 1.0, scalar2=None, op0=OP.add)
                nc.vector.tensor_mul(jf, jf, maskf)
                nc.vector.tensor_scalar(outi[:, rt:rt + 1], jf, 1.0, scalar2=None,
                                        op0=OP.subtract)

            # ---------------- outputs ---------------------------------------
            outi_i = cpool.tile([P, RT], I32, tag="outi_i")
            nc.vector.tensor_copy(outi_i, outi)
            outm_u = cpool.tile([P, RT], U8, tag="outm_u")
            nc.vector.tensor_copy(outm_u, outm)
            nc.sync.dma_start(out=o_ratio[:, :], in_=outr)
            nc.sync.dma_start(out=o_idx[:, :], in_=outi_i)
            nc.sync.dma_start(out=o_mask[:, :], in_=outm_u)

    nc.finalize()
    return nc


def _get_nc():
    if "nc" not in _CACHE:
        _CACHE["nc"] = build_nc()
    return _CACHE["nc"]


def kernel(desc1, desc2, _trace=False, _tmpdir=None):
    desc1 = np.ascontiguousarray(np.asarray(desc1, dtype=np.float32))
    desc2 = np.ascontiguousarray(np.asarray(desc2, dtype=np.float32))
    assert desc1.shape == (B1, D) and desc2.shape == (B2, D)

    d2t = np.ascontiguousarray(desc2.T)
    in_maps = []
    for c in range(NCORES):
        sl = slice(c * ROWS_PER_CORE, (c + 1) * ROWS_PER_CORE)
        d1n_c = np.ascontiguousarray(desc1[sl])
        d1t_c = np.ascontiguousarray(d1n_c.T)
        in_maps.append({"d1n": d1n_c, "d1t": d1t_c, "d2t": d2t, "d2n": desc2})

    nc = _get_nc()
    res = run_bass_kernel_spmd(
        nc, in_maps, core_ids=list(range(NCORES)),
        trace=_trace, tmpdir=_tmpdir,
    )
    if _trace:
        _CACHE["last_result"] = res

    ratios, idxs, masks = [], [], []
    for c in range(NCORES):
        r = res.results[c]
        ratios.append(r["o_ratio"].T.reshape(-1))       # (RT,P)->(1024,)
        idxs.append(r["o_idx"].T.reshape(-1))
        masks.append(r["o_mask"].T.reshape(-1))
    ratio = np.concatenate(ratios).astype(np.float32)
    idx1 = np.concatenate(idxs).astype(np.int32)
    mask = np.concatenate(masks).astype(bool)

    match_dists = ratio[:, None]
    col0 = np.where(mask, np.arange(B1, dtype=np.int32), np.int32(-1))
    matches_idxs = np.stack([col0, idx1], axis=1).astype(np.int32)
    return match_dists, matches_idxs, mask


# revision 8
# speedup vs baseline: 1.0351x; 1.0351x over previous
"""DescriptorMatcher (SNN ratio-test) Trainium2 kernel.

Problem: desc1, desc2 (8192, 128) f32. For each row i of desc1: find the two
nearest neighbours in desc2 (L2), ratio = d1/d2, mask = ratio <= 0.8, emit
(ratio-or-0, [i, argmin]-or--1, mask).

Sharding: desc1 rows split across 8 cores (1024 rows each), desc2 replicated.

Per-core algorithm (all on device):
  - Build bf16 hi/lo splits of d1-slice^T and desc2^T.
  - s = 2*d1@d2^T - ||d2||^2  (negated squared distance + per-row const),
    computed per (128 rows x 512 cols) chunk as 3 bf16 matmul passes
    (hi*hi + hi*lo + lo*hi, scaled by 2 via pre-scaled d1 tiles) plus a K=2
    "c-fold" matmul adding -||d2||^2 (split hi/lo in bf16).
  - ScalarE copies PSUM chunks into an SBUF row-tile s (128 x 8192).
  - VectorE max8 -> top-8 values; find_index8 -> their column indices.
  - Exact refinement: indirect-DMA gather desc2[j1], desc2[j2]; recompute
    both distances exactly in fp32; ratio = sqrt(d1sq/d2sq) (ACT sqrt +
    one Newton step); mask; masked outputs.
Outputs per core: ratio (128,8) f32, idx (128,8) i32, mask (128,8) u8,
column rt = row-tile rt (rows rt*128..rt*128+127 of the core's slice).
"""

import numpy as np

import concourse.bass as bass
import concourse.mybir as mybir
from concourse.bacc import Bacc
from concourse.tile import TileContext
from concourse.bass_utils import run_bass_kernel_spmd

P = 128          # partitions / rows per row-tile
B1 = 8192        # rows of desc1
B2 = 8192        # rows of desc2
D = 128          # descriptor dim
NCORES = 8
ROWS_PER_CORE = B1 // NCORES       # 1024
RT = ROWS_PER_CORE // P            # 8 row-tiles per core
N = 512                            # matmul free-dim chunk
NCH = B2 // N                      # 16 chunks
TH = 0.8

F32 = mybir.dt.float32
BF16 = mybir.dt.bfloat16
I32 = mybir.dt.int32
U8 = mybir.dt.uint8
U32 = mybir.dt.uint32
AF = mybir.ActivationFunctionType
OP = mybir.AluOpType

_CACHE = {}


def build_nc(stage=99, nrt=RT):
    nc = Bacc()
    d1n = nc.dram_tensor("d1n", [ROWS_PER_CORE, D], F32, kind="ExternalInput")
    d1t = nc.dram_tensor("d1t", [D, ROWS_PER_CORE], F32, kind="ExternalInput")
    d2t = nc.dram_tensor("d2t", [D, B2], F32, kind="ExternalInput")
    d2n = nc.dram_tensor("d2n", [B2, D], F32, kind="ExternalInput")
    o_ratio = nc.dram_tensor("o_ratio", [P, RT], F32, kind="ExternalOutput")
    o_idx = nc.dram_tensor("o_idx", [P, RT], I32, kind="ExternalOutput")
    o_mask = nc.dram_tensor("o_mask", [P, RT], U8, kind="ExternalOutput")

    with TileContext(nc) as tc:
        with tc.tile_pool(name="const", bufs=1) as cpool, \
             tc.tile_pool(name="load", bufs=3) as ldpool, \
             tc.tile_pool(name="setup", bufs=2) as supool, \
             tc.tile_pool(name="scan", bufs=3) as spool, \
             tc.tile_pool(name="small", bufs=3) as smpool, \
             tc.tile_pool(name="ps_set", bufs=2, space="PSUM") as psB, \
             tc.tile_pool(name="ps_mm", bufs=5, space="PSUM") as psA:

            # ---------------- constants / persistent tiles ----------------
            d2hi = cpool.tile([P, B2], BF16, tag="d2hi")
            d2lo = cpool.tile([P, B2], BF16, tag="d2lo")
            cf = cpool.tile([2, B2], BF16, tag="cf")       # rows: -d2sq hi, lo
            d1hi = cpool.tile([P, ROWS_PER_CORE], BF16, tag="d1hi")
            d1lo = cpool.tile([P, ROWS_PER_CORE], BF16, tag="d1lo")
            mones = cpool.tile([P, P], F32, tag="mones")   # -1.0 weights
            ones2 = cpool.tile([2, P], BF16, tag="ones2")
            outr = cpool.tile([P, RT], F32, tag="outr")
            outi = cpool.tile([P, RT], F32, tag="outi")    # staged as f32
            outm = cpool.tile([P, RT], F32, tag="outm")

            nc.gpsimd.memset(mones, -1.0)
            nc.gpsimd.memset(ones2, 1.0)
            nc.gpsimd.memset(outr, 0.0)
            nc.gpsimd.memset(outi, -1.0)
            nc.gpsimd.memset(outm, 0.0)

            # ---------------- setup: d1 splits (x2 for the "2*" in 2ab) ----
            d1f = cpool.tile([P, ROWS_PER_CORE], F32, tag="d1f")
            nc.sync.dma_start(out=d1f, in_=d1t[:, :])
            d1x2 = cpool.tile([P, ROWS_PER_CORE], F32, tag="d1x2")
            nc.vector.tensor_scalar(d1x2, d1f, 2.0, scalar2=None, op0=OP.mult)
            nc.scalar.copy(d1hi, d1x2)
            d1lo32 = cpool.tile([P, ROWS_PER_CORE], F32, tag="d1lo32")
            nc.gpsimd.tensor_sub(d1lo32, d1x2, d1hi)
            nc.gpsimd.tensor_copy(d1lo, d1lo32)

            # ---------------- setup: d2 splits + c rows, per chunk ---------
            for n in range(NCH):
                sl = bass.ts(n, N)
                d2c = ldpool.tile([P, N], F32, tag="d2c")
                nc.sync.dma_start(out=d2c, in_=d2t[:, sl])
                nc.scalar.copy(d2hi[:, sl], d2c)
                d2lo32 = supool.tile([P, N], F32, tag="d2lo32")
                nc.gpsimd.tensor_sub(d2lo32, d2c, d2hi[:, sl])
                nc.gpsimd.tensor_copy(d2lo[:, sl], d2lo32)
                # c row: -sum(d2^2) replicated over partitions via matmul
                sq = supool.tile([P, N], F32, tag="sq")
                nc.scalar.activation(sq, d2c, AF.Square)
                cps = psB.tile([P, N], F32, tag="csq")
                nc.tensor.matmul(cps, mones, sq, start=True, stop=True)
                chi = supool.tile([P, N], BF16, tag="chi")
                nc.scalar.copy(chi, cps)
                crep = supool.tile([P, N], F32, tag="crep")
                nc.scalar.copy(crep, cps)
                crem = supool.tile([P, N], F32, tag="crem")
                nc.gpsimd.tensor_sub(crem, crep, chi)
                clo = supool.tile([P, N], BF16, tag="clo")
                nc.gpsimd.tensor_copy(clo, crem)
                # compute engines cannot write at partition base 1 -> DMA rows in
                nc.sync.dma_start(out=cf[0:1, sl], in_=chi[0:1, :])
                nc.sync.dma_start(out=cf[1:2, sl], in_=clo[0:1, :])

            # ---------------- steady: per row-tile --------------------------
            for rt in range(nrt if stage >= 2 else 0):
                rsl = bass.ts(rt, P)
                l_hi = d1hi[:, rsl]
                l_lo = d1lo[:, rsl]
                d1nc = ldpool.tile([P, D], F32, tag="d1nc")
                nc.sync.dma_start(out=d1nc, in_=d1n[rsl, :])

                s_tile = spool.tile([P, B2], F32, tag="s")
                for n in range(NCH):
                    sl = bass.ts(n, N)
                    ps = psA.tile([P, N], F32, tag="mm")
                    nc.tensor.matmul(ps, l_hi, d2hi[:, sl], start=True, stop=False)
                    nc.tensor.matmul(ps, l_hi, d2lo[:, sl], start=False, stop=False)
                    nc.tensor.matmul(ps, l_lo, d2hi[:, sl], start=False, stop=False)
                    nc.tensor.matmul(ps, ones2, cf[:, sl], start=False, stop=True)
                    nc.scalar.copy(s_tile[:, sl], ps)

                if stage < 3:
                    continue
                m8 = smpool.tile([P, 8], F32, tag="m8")
                nc.vector.max(out=m8, in_=s_tile)
                i8 = smpool.tile([P, 8], U32, tag="i8")
                nc.vector.max_index(out=i8, in_max=m8, in_values=s_tile)

                if stage < 4:
                    continue
                # exact refinement: gather desc2 rows for top-2 indices
                g1 = smpool.tile([P, D], F32, tag="g1")
                nc.gpsimd.indirect_dma_start(
                    out=g1, out_offset=None, in_=d2n[:, :],
                    in_offset=bass.IndirectOffsetOnAxis(ap=i8[:, 0:1], axis=0))
                g2 = smpool.tile([P, D], F32, tag="g2")
                nc.gpsimd.indirect_dma_start(
                    out=g2, out_offset=None, in_=d2n[:, :],
                    in_offset=bass.IndirectOffsetOnAxis(ap=i8[:, 1:2], axis=0))

                if stage < 5:
                    continue
                t1 = smpool.tile([P, D], F32, tag="t1")
                nc.gpsimd.tensor_sub(t1, g1, d1nc)
                t2 = smpool.tile([P, D], F32, tag="t2")
                nc.gpsimd.tensor_sub(t2, g2, d1nc)
                sc1 = smpool.tile([P, D], F32, tag="sc1")
                nc.gpsimd.tensor_mul(sc1, t1, t1)
                dist1 = smpool.tile([P, 1], F32, tag="dist1")
                nc.vector.reduce_sum(dist1, sc1, axis=mybir.AxisListType.X)
                sc2 = smpool.tile([P, D], F32, tag="sc2")
                nc.gpsimd.tensor_mul(sc2, t2, t2)
                dist2 = smpool.tile([P, 1], F32, tag="dist2")
                nc.vector.reduce_sum(dist2, sc2, axis=mybir.AxisListType.X)

                # ratio = sqrt(dist1/dist2), one Newton step for the sqrt
                if stage < 6:
                    continue
                rec = smpool.tile([P, 1], F32, tag="rec")
                nc.vector.reciprocal(rec, dist2)
                q = smpool.tile([P, 1], F32, tag="q")
                nc.vector.tensor_mul(q, dist1, rec)
                if stage < 7:
                    continue
                r0 = smpool.tile([P, 1], F32, tag="r0")
                nc.scalar.activation(r0, q, AF.Sqrt)
                rr = smpool.tile([P, 1], F32, tag="rr")
                nc.vector.reciprocal(rr, r0)
                t = smpool.tile([P, 1], F32, tag="t")
                nc.vector.tensor_mul(t, q, rr)
                r1 = smpool.tile([P, 1], F32, tag="r1")
                nc.vector.tensor_add(r1, r0, t)
                nc.vector.tensor_scalar(r1, r1, 0.5, scalar2=None, op0=OP.mult)

                if stage < 8:
                    continue
                maskf = smpool.tile([P, 1], F32, tag="maskf")
                nc.vector.tensor_scalar(maskf, r1, TH, scalar2=None, op0=OP.is_le)
                nc.vector.tensor_mul(outr[:, rt:rt + 1], r1, maskf)
                nc.vector.tensor_copy(outm[:, rt:rt + 1], maskf)

                if stage < 9:
                    continue
                jf = smpool.tile([P, 1], F32, tag="jf")
                nc.vector.tensor_copy(jf, i8[:, 0:1])
                nc.vector.tensor_scalar(jf, jf, 1.0, scalar2=None, op0=OP.add)
                nc.vector.tensor_mul(jf, jf, maskf)
                nc.vector.tensor_scalar(outi[:, rt:rt + 1], jf, 1.0, scalar2=None,
                                        op0=OP.subtract)

            # ---------------- outputs ---------------------------------------
            outi_i = cpool.tile([P, RT], I32, tag="outi_i")
            nc.vector.tensor_copy(outi_i, outi)
            outm_u = cpool.tile([P, RT], U8, tag="outm_u")
            nc.vector.tensor_copy(outm_u, outm)
            nc.sync.dma_start(out=o_ratio[:, :], in_=outr)
            nc.sync.dma_start(out=o_idx[:, :], in_=outi_i)
            nc.sync.dma_start(out=o_mask[:, :], in_=outm_u)

    nc.finalize()
    return nc


def _get_nc():
    if "nc" not in _CACHE:
        _CACHE["nc"] = build_nc()
    return _CACHE["nc"]


def kernel(desc1, desc2, _trace=False, _tmpdir=None):
    desc1 = np.ascontiguousarray(np.asarray(desc1, dtype=np.float32))
    desc2 = np.ascontiguousarray(np.asarray(desc2, dtype=np.float32))
    assert desc1.shape == (B1, D) and desc2.shape == (B2, D)

    d2t = np.ascontiguousarray(desc2.T)
    in_maps = []
    for c in range(NCORES):
        sl = slice(c * ROWS_PER_CORE, (c + 1) * ROWS_PER_CORE)
        d1n_c = np.ascontiguousarray(desc1[sl])
        d1t_c = np.ascontiguousarray(d1n_c.T)
        in_maps.append({"d1n": d1n_c, "d1t": d1t_c, "d2t": d2t, "d2n": desc2})

    nc = _get_nc()
    res = run_bass_kernel_spmd(
        nc, in_maps, core_ids=list(range(NCORES)),
        trace=_trace, tmpdir=_tmpdir,
    )
    if _trace:
        _CACHE["last_result"] = res

    ratios, idxs, masks = [], [], []
    for c in range(NCORES):
        r = res.results[c]
        ratios.append(r["o_ratio"].T.reshape(-1))       # (RT,P)->(1024,)
        idxs.append(r["o_idx"].T.reshape(-1))
        masks.append(r["o_mask"].T.reshape(-1))
    ratio = np.concatenate(ratios).astype(np.float32)
    idx1 = np.concatenate(idxs).astype(np.int32)
    mask = np.concatenate(masks).astype(bool)

    match_dists = ratio[:, None]
    col0 = np.where(mask, np.arange(B1, dtype=np.int32), np.int32(-1))
    matches_idxs = np.stack([col0, idx1], axis=1).astype(np.int32)
    return match_dists, matches_idxs, mask
